# revision 24
# baseline (speedup 1.0000x reference)
"""Trainium2 Bass kernel for causal multi-head attention + output projection.

Problem: B=2, S=2048, D=1024, H=16 heads of HD=64; fp32; causal softmax
scaled by D**-0.5; output projection with bias.

Sharding: 2 heads per core (tensor parallel on heads) for QKV + attention,
then an on-device AllToAll reshards from head-split to sequence-split and
each core computes its 512 rows of the output projection locally.

Math notes:
 - All attention tensors are kept transposed ([feature, seq] layouts) so
   every matmul contracts on the partition dim with zero on-chip transposes
   (except V, which is produced as V^T and transposed via the PE).
 - softmax is computed without max-subtraction: logits are N(0, 1/16) by
   construction (scale = 1/32 over a 64-dim dot of unit-variance q,k), so
   exp() is numerically safe; the denominator is accumulated by a column of
   ones appended to V (row 64 of the O^T PSUM accumulator).
 - float32r (TF32-like) matmuls run at bf16 rate with ~1e-4 relative error.
"""

import sys

sys.path.insert(0, "/opt/trn_rl_repo")

import numpy as np

import concourse.bacc as bacc
import concourse.mybir as mybir
import concourse.tile as tile
from concourse.bass_utils import run_bass_kernel_spmd
B, D, H, HD = 2, 1024, 16, 64
NCORES = 8
SCALE = float(D) ** -0.5
F32 = mybir.dt.float32
F32R = mybir.dt.float32r
BF16 = mybir.dt.bfloat16
Exp = mybir.ActivationFunctionType.Exp


def build(S=2048, dump=False):
    KD = D // 128          # contraction tiles for the projections
    NT = S // 128          # key tiles
    SQ = 512               # query-chunk width
    NCH = S // SQ          # query chunks per (batch, head)
    HSL = S // NCORES      # rows of output owned per core per batch

    nc = bacc.Bacc("TRN2", target_bir_lowering=False, debug=False)
    xT = nc.dram_tensor("xT", [B, D, S], BF16, kind="ExternalInput")
    Wqkv = nc.dram_tensor("Wqkv", [128, 3, D // 128, 128], BF16, kind="ExternalInput")
    WpT = nc.dram_tensor("WpT", [128, D // 128, D], BF16, kind="ExternalInput")
    bp = nc.dram_tensor("bp", [1, D], BF16, kind="ExternalInput")
    mask = nc.dram_tensor("mask", [128, 128], BF16, kind="ExternalInput")
    idin = nc.dram_tensor("idin", [128, 128], BF16, kind="ExternalInput")
    sel = nc.dram_tensor("sel", [16, KD, 128], F32, kind="ExternalInput")
    # y rows: [0:HSL] = batch0 s-slice, [HSL:2*HSL] = batch1 s-slice
    y = nc.dram_tensor("y", [B * HSL, D], F32, kind="ExternalOutput")

    with tile.TileContext(nc) as tc:
        ctx_pools = [
            tc.tile_pool(name="persist", bufs=1),
            tc.tile_pool(name="dram", bufs=1, space="DRAM"),
            tc.tile_pool(name="wq", bufs=1),
            tc.tile_pool(name="xp", bufs=2),
            tc.tile_pool(name="qk", bufs=2),
            tc.tile_pool(name="vp", bufs=2),
            tc.tile_pool(name="at", bufs=6),
            tc.tile_pool(name="prj", bufs=2),
            tc.tile_pool(name="yo", bufs=2),
            tc.tile_pool(name="ps_mix", bufs=2, space="PSUM"),
            tc.tile_pool(name="ps_sc", bufs=2, space="PSUM"),
            tc.tile_pool(name="ps_oT", bufs=2, space="PSUM"),
        ]
        import contextlib

        with contextlib.ExitStack() as stk:
            (
                persist, dram, wpool, xpool, qkpool, vppool, atpool,
                prjpool, ypool, ps_mix, ps_sc, ps_oT,
            ) = [stk.enter_context(p) for p in ctx_pools]

            # ---- critical-path first: small constants, weights, batch-0 x ----
            ident = persist.tile([128, 128], BF16)
            nc.sync.dma_start(out=ident, in_=idin[:, :])
            mask_sb = persist.tile([128, 128], BF16)
            nc.sync.dma_start(out=mask_sb, in_=mask[:, :])
            wqkv_sb = wpool.tile([128, 3, KD, 128], BF16)
            nc.sync.dma_start(out=wqkv_sb, in_=Wqkv[:, :, :, :])
            # PE warm-up while input DMAs are in flight: ramps the clock gate
            wps = ps_sc.tile([128, 2, SQ], F32, tag="ps_sc", name="warmps")
            for _ in range(90):
                nc.tensor.matmul(wps[:, 0, 0:128], ident, ident, start=True, stop=True)

            def load_x(b):
                xs = [
                    xpool.tile([128, S], BF16, tag=f"x{t}", name=f"x_{b}_{t}")
                    for t in range(KD)
                ]
                for t in range(KD):
                    nc.sync.dma_start(
                        out=xs[t], in_=xT[b, 128 * t : 128 * (t + 1), :]
                    )
                return xs

            x_sb = {0: load_x(0)}

            ones_sb = persist.tile([1, 128], BF16)
            nc.vector.memset(ones_sb, 1.0)

            oT_sb = {
                (b, hs): persist.tile(
                    [65, S], F32, tag=f"oT_{b}_{hs}", name=f"oT_{b}_{hs}"
                )
                for b in range(B)
                for hs in range(2)
            }
            a2a_in = {
                (b, hs): dram.tile([NCORES, 65, HSL], F32, name=f"a2a_in_{b}_{hs}")
                for b in range(B)
                for hs in range(2)
            }
            a2a_out = {
                (b, hs): dram.tile([NCORES, 65, HSL], F32, name=f"a2a_out_{b}_{hs}")
                for b in range(B)
                for hs in range(2)
            }
            qkvT = {}
            vp = {}

            def emit_qkv_group(b, w, n, eng=None):
                if b not in qkvT:
                    qkvT[b] = qkpool.tile(
                        [128, 3, S], BF16, tag="qkvT", name=f"qkvT_{b}"
                    )
                ps = ps_mix.tile([128, SQ], F32, tag="mix", name=f"psqk_{b}_{w}_{n}")
                for t in range(KD):
                    nc.tensor.matmul(
                        ps,
                        wqkv_sb[:, w, t, :],
                        x_sb[b][t][:, SQ * n : SQ * (n + 1)],
                        start=(t == 0),
                        stop=(t == KD - 1),
                    )
                dst = qkvT[b][:, w, SQ * n : SQ * (n + 1)]
                if eng == "scalar":
                    nc.scalar.copy(dst, ps)
                else:
                    nc.vector.tensor_copy(dst, ps)

            def emit_v_unit(b, i):
                if b not in vp:
                    vp[b] = vppool.tile(
                        [128, NT, 2, 65], BF16, tag="vp", name=f"vp_{b}"
                    )
                    nc.vector.memset(vp[b][:, :, :, 64], 1.0)
                pst = ps_mix.tile([128, 128], BF16, tag="mix", name=f"psvt_{b}_{i}")
                nc.tensor.transpose(
                    pst, qkvT[b][:, 2, 128 * i : 128 * (i + 1)], ident[:, :]
                )
                for hs in range(2):
                    nc.vector.tensor_copy(
                        vp[b][:, i, hs, 0:64], pst[:, 64 * hs : 64 * hs + 64]
                    )

            def emit_attn_chunk(b, hs, n, fillers, stride=2):
                qT = qkvT[b][64 * hs : 64 * hs + 64, 0, :]
                kT = qkvT[b][64 * hs : 64 * hs + 64, 1, :]
                ot = ps_oT.tile(
                    [65, SQ], F32, tag="ps_oT", name=f"ot_{b}_{hs}_{n}"
                )
                jmax = 4 * n + 4
                for jp in range(0, jmax, 2):
                    sc = ps_sc.tile(
                        [128, 2, SQ], F32, tag="ps_sc", name=f"sc_{b}_{hs}_{n}_{jp}"
                    )
                    at = atpool.tile([128, 2, SQ], BF16, tag="at")
                    offs = []
                    for k in range(2):
                        j = jp + k
                        off = max(0, 128 * j - SQ * n)
                        offs.append(off)
                        nc.tensor.matmul(
                            sc[:, k, off:],
                            kT[:, 128 * j : 128 * (j + 1)],
                            qT[:, SQ * n + off : SQ * (n + 1)],
                            start=True,
                            stop=True,
                        )
                    # one exp over both halves (covers a dead zone between them)
                    o0 = offs[0]
                    nc.scalar.activation(
                        at[:, :, :].rearrange("p a s -> p (a s)")[:, o0:],
                        sc[:, :, :].rearrange("p a s -> p (a s)")[:, o0:],
                        Exp,
                        scale=SCALE,
                    )
                    for k in range(2):
                        j = jp + k
                        off = offs[k]
                        if j >= 4 * n:
                            nc.gpsimd.tensor_mul(
                                at[:, k, off : off + 128],
                                at[:, k, off : off + 128],
                                mask_sb,
                            )
                        nc.tensor.matmul(
                            ot[:, off:],
                            vp[b][:, j, hs, :],
                            at[:, k, off:],
                            start=(j == 0),
                            stop=(j == jmax - 1),
                        )
                    if fillers:
                        fillers[0] -= 2
                        if fillers[0] <= 0 and len(fillers) > 1:
                            fillers[0] = fillers.pop(1)
                            fillers.pop(1)()
                nc.vector.tensor_copy(oT_sb[(b, hs)][:, SQ * n : SQ * (n + 1)], ot)

            def emit_staging_cc(b, hs):
                for d in range(NCORES):
                    s0 = d * HSL
                    nc.sync.dma_start(
                        out=a2a_in[(b, hs)][d, :, :],
                        in_=oT_sb[(b, hs)][:, s0 : s0 + HSL],
                    )
                nc.gpsimd.collective_compute(
                    "AllToAll",
                    mybir.AluOpType.bypass,
                    replica_groups=[list(range(NCORES))],
                    ins=[a2a_in[(b, hs)][:, :, :].opt()],
                    outs=[a2a_out[(b, hs)][:, :, :].opt()],
                )

            # ---- phase D (output projection) pieces for batch b ----
            dstate = {}

            def emit_D_head(b):
                st_ = {}
                st_["den"] = prjpool.tile([16, HSL], F32, tag="den", name=f"den_{b}")
                for h_ in range(2):
                    nc.gpsimd.dma_start(
                        out=st_["den"][8 * h_ : 8 * h_ + 8, :],
                        in_=a2a_out[(b, h_)][:, 64, :],
                    )
                st_["rcp"] = prjpool.tile([16, HSL], F32R, tag="rcp", name=f"rcp_{b}")
                with nc.allow_low_precision(reason="softmax denom recip"):
                    nc.vector.reciprocal(st_["rcp"], st_["den"][:, :].bitcast(F32R))
                st_["onrm"] = prjpool.tile(
                    [128, KD, HSL], BF16, tag="onrm", name=f"onrm_{b}"
                )
                st_["accs"] = {}
                dstate[b] = st_

            def emit_D_norm(b, t):
                st_ = dstate[b]
                orc = prjpool.tile([128, HSL], F32, tag="orc", name=f"orc_{b}_{t}")
                for h_ in range(2):
                    nc.sync.dma_start(
                        out=orc[64 * h_ : 64 * h_ + 64, :],
                        in_=a2a_out[(b, h_)][t, 0:64, :],
                    )
                bc = ps_mix.tile([128, HSL], F32, tag="mix", name=f"bc_{b}_{t}")
                nc.tensor.matmul(bc, sel_sb[:, t, :], st_["rcp"], start=True, stop=True)
                nc.vector.tensor_mul(st_["onrm"][:, t, :], orc, bc)

            def emit_D_group(b, st, nn):
                st_ = dstate[b]
                acc = ps_mix.tile(
                    [128, 512], F32, tag="mix", name=f"acc_{b}_{st}_{nn}"
                )
                for t in range(KD):
                    nc.tensor.matmul(
                        acc,
                        st_["onrm"][:, t, 128 * st : 128 * (st + 1)],
                        wpT_sb[:, t, 512 * nn : 512 * (nn + 1)],
                        start=(t == 0),
                        stop=False,
                    )
                nc.tensor.matmul(
                    acc, ones_sb, bp_sb[:, 512 * nn : 512 * (nn + 1)],
                    start=False, stop=True,
                )
                ys = st_.setdefault("y", {})
                if st not in ys:
                    ys[st] = ypool.tile(
                        [128, D], F32, tag="y", name=f"y_{b}_{st}"
                    )
                if b == 1:
                    nc.scalar.copy(ys[st][:, 512 * nn : 512 * (nn + 1)], acc)
                else:
                    nc.vector.tensor_copy(ys[st][:, 512 * nn : 512 * (nn + 1)], acc)
                if nn == D // 512 - 1:
                    nc.sync.dma_start(
                        out=y[b * HSL + 128 * st : b * HSL + 128 * (st + 1), :],
                        in_=ys[st],
                    )

            # ---- schedule ----
            # batch 0 projections (dense PE, warms HAM)
            for w in range(3):
                for n in range(NCH):
                    emit_qkv_group(0, w, n, eng="scalar")
            for i in range(NT):
                emit_v_unit(0, i)
            # batch-1 x load starts now (behind batch-0 x on the DMA queues)
            x_sb[1] = load_x(1)

            # batch-0 attention with batch-1 projection woven in
            units = []
            for w in range(3):
                for n in range(NCH):
                    units.append(lambda w=w, n=n: emit_qkv_group(1, w, n))
            for i in range(NT):
                units.append(lambda i=i: emit_v_unit(1, i))
            total_j = 2 * sum(4 * n + 4 for n in range(NCH))
            pace = max(2, total_j // max(len(units), 1))
            fillers = [pace]
            for u in units:
                fillers.extend([pace, u])
            for hs in range(2):
                for n in range(NCH):
                    emit_attn_chunk(0, hs, n, fillers)
                emit_staging_cc(0, hs)
            for k in range(2, len(fillers), 2):
                fillers[k]()

            # deferred big loads for phase D (transfer during batch-1 attention)
            wpT_sb = persist.tile([128, KD, D], BF16)
            nc.sync.dma_start(out=wpT_sb, in_=WpT[:, :, :])
            bp_sb = persist.tile([1, D], BF16)
            nc.sync.dma_start(out=bp_sb, in_=bp[:, :])
            sel_sb = persist.tile([16, KD, 128], F32R)
            nc.sync.dma_start(out=sel_sb, in_=sel[:, :, :].bitcast(F32R))

            # batch-1 attention with phase-D(batch 0) woven in
            units = [lambda: emit_D_head(0)]
            for t in range(KD):
                units.append(lambda t=t: emit_D_norm(0, t))
            for st in range(HSL // 128):
                for nn in range(D // 512):
                    units.append(lambda st=st, nn=nn: emit_D_group(0, st, nn))
            chunks1 = [(hs, n) for hs in range(2) for n in range(NCH)]
            start_ci = len(chunks1) // 2 + 1
            late = chunks1[start_ci:]
            total_j = 2 * sum(4 * n + 4 for _, n in late)
            pace = max(2, total_j // max(len(units) - 1, 1))
            fillers = [pace]
            for u in units[1:]:
                fillers.extend([pace, u])
            for ci, (hs, n) in enumerate(chunks1):
                if ci == start_ci - 1:
                    units[0]()  # D0 head: den DMAs + reciprocal
                emit_attn_chunk(1, hs, n, fillers if ci >= start_ci else None)
                if n == NCH - 1:
                    emit_staging_cc(1, hs)
            for k in range(2, len(fillers), 2):
                fillers[k]()

            # phase D for batch 1 (tail)
            emit_D_head(1)
            for t in range(KD):
                emit_D_norm(1, t)
            for st in range(HSL // 128):
                for nn in range(D // 512):
                    emit_D_group(1, st, nn)

    nc.compile()
    return nc


_built = {}


def get_nc(S=2048):
    if S not in _built:
        _built[S] = build(S)
    return _built[S]


def prep_inputs(x, Wq, Wk, Wv, Wp, bp):
    """Host-side shard prep. Returns per-core input maps."""
    import ml_dtypes

    BF = ml_dtypes.bfloat16
    x = np.ascontiguousarray(np.asarray(x, dtype=np.float32))
    Wq, Wk, Wv = (np.asarray(w, dtype=np.float32) for w in (Wq, Wk, Wv))
    Wp = np.asarray(Wp, dtype=np.float32)
    bp = np.asarray(bp, dtype=np.float32)
    BFc = BF
    xT = np.ascontiguousarray(x.transpose(0, 2, 1)).astype(BFc)
    KD = D // 128
    # WpT pre-arranged for SBUF: [p, t, i] with row t*128+p of Wp.T
    WpT = np.ascontiguousarray(
        Wp.T.reshape(KD, 128, D).transpose(1, 0, 2)
    ).astype(BFc)
    mask = np.triu(np.ones((128, 128), dtype=np.float32)).astype(BFc)
    idin = np.eye(128, dtype=np.float32).astype(BFc)
    sel = np.zeros((16, KD, 128), dtype=np.float32)
    for t in range(KD):
        sel[t, t, 0:64] = 1.0          # head 2t     -> den row 0*8 + t
        sel[8 + t, t, 64:128] = 1.0    # head 2t + 1 -> den row 1*8 + t
    in_maps = []
    for c in range(NCORES):
        h0 = 2 * c
        wqkv = np.stack(
            [
                np.concatenate([Wq[h0], Wq[h0 + 1]], axis=1),
                np.concatenate([Wk[h0], Wk[h0 + 1]], axis=1),
                np.concatenate([Wv[h0], Wv[h0 + 1]], axis=1),
            ]
        )  # [3, D, 128]
        # pre-arrange: [p, w, t, m]
        wqkv = np.ascontiguousarray(
            wqkv.reshape(3, KD, 128, 128).transpose(2, 0, 1, 3)
        ).astype(BF)
        in_maps.append(
            {
                "xT": xT,
                "Wqkv": wqkv,
                "WpT": WpT,
                "bp": bp.reshape(1, D).astype(BF),
                "mask": mask,
                "idin": idin,
                "sel": sel,
            }
        )
    return in_maps


# inputs identical across cores are passed replicated (shipped once, not 8x)
_REPLICATED = {"xT", "WpT", "bp", "mask", "idin"}

_runners = {}


def _get_runner(S):
    """Cached jitted SPMD callable for the built module."""
    if S in _runners:
        return _runners[S]
    import jax
    import concourse.mybir as _mybir
    from concourse import bass2jax
    from jax.experimental.shard_map import shard_map
    from jax.sharding import Mesh, PartitionSpec

    nc = get_nc(S)
    bass2jax.install_neuronx_cc_hook()

    in_names, out_names, out_avals = [], [], []
    partition_name = nc.partition_id_tensor.name if nc.partition_id_tensor else None
    for alloc in nc.m.functions[0].allocations:
        if not isinstance(alloc, _mybir.MemoryLocationSet):
            continue
        name = alloc.memorylocations[0].name
        if alloc.kind == "ExternalInput":
            if name != partition_name:
                in_names.append(name)
        elif alloc.kind == "ExternalOutput":
            out_names.append(name)
            out_avals.append(
                jax.core.ShapedArray(tuple(alloc.tensor_shape), _mybir.dt.np(alloc.dtype))
            )
    n_params = len(in_names)
    all_in_names = list(in_names) + list(out_names)
    if partition_name is not None:
        all_in_names.append(partition_name)

    def _body(*args):
        operands = list(args)
        if partition_name is not None:
            operands.append(bass2jax.partition_id_tensor())
        outs = bass2jax._bass_exec_p.bind(
            *operands,
            out_avals=tuple(out_avals),
            in_names=tuple(all_in_names),
            out_names=tuple(out_names),
            lowering_input_output_aliases=(),
            sim_require_finite=True,
            sim_require_nnan=True,
            nc=nc,
        )
        return tuple(outs)

    devices = jax.devices()[:NCORES]
    mesh = Mesh(np.asarray(devices), ("core",))
    in_specs = tuple(
        PartitionSpec() if nm in _REPLICATED else PartitionSpec("core")
        for nm in in_names
    ) + (PartitionSpec("core"),) * len(out_names)
    out_specs = (PartitionSpec("core"),) * len(out_names)
    donate = tuple(range(n_params, n_params + len(out_names)))
    fn = jax.jit(
        shard_map(_body, mesh=mesh, in_specs=in_specs, out_specs=out_specs, check_rep=False),
        donate_argnums=donate,
        keep_unused=True,
    )
    r = (fn, in_names, out_names, out_avals, mesh)
    _runners[S] = r
    return r


class _Res:
    def __init__(self, results):
        self.results = results
        self.exec_time_ns = None


def run(x, Wq, Wk, Wv, Wp, bp, timings=None):
    import time as _time

    S = x.shape[1]
    t0 = _time.perf_counter()
    fn, in_names, out_names, out_avals, mesh = _get_runner(S)
    t1 = _time.perf_counter()
    in_maps = prep_inputs(x, Wq, Wk, Wv, Wp, bp)
    t2 = _time.perf_counter()
    args = []
    for nm in in_names:
        if nm in _REPLICATED:
            args.append(in_maps[0][nm])
        else:
            args.append(np.concatenate([in_maps[c][nm] for c in range(NCORES)], axis=0))
    zero_outs = [
        np.zeros((NCORES * av.shape[0], *av.shape[1:]), av.dtype) for av in out_avals
    ]
    t3 = _time.perf_counter()
    out_arrs = fn(*args, *zero_outs)
    out_np = [np.asarray(o) for o in out_arrs]
    t4 = _time.perf_counter()
    results = [
        {
            nm: out_np[i].reshape(NCORES, *out_avals[i].shape)[c]
            for i, nm in enumerate(out_names)
        }
        for c in range(NCORES)
    ]
    if timings is not None:
        timings.update(
            runner=t1 - t0, prep=t2 - t1, concat=t3 - t2, exec=t4 - t3
        )
    return _assemble_y([results[c]["y"] for c in range(NCORES)]), _Res(results)


def _assemble_y(per_core):
    """per-core y is [B*HSL, D]: rows [b*HSL:(b+1)*HSL] = batch b, s-slice c."""
    HSL = per_core[0].shape[0] // B
    S = HSL * NCORES
    out = np.empty((B, S, D), dtype=per_core[0].dtype)
    for c in range(NCORES):
        for b in range(B):
            out[b, HSL * c : HSL * (c + 1), :] = per_core[c][b * HSL : (b + 1) * HSL]
    return out


def kernel(x, Wq, Wk, Wv, Wp, bp):
    out, _ = run(x, Wq, Wk, Wv, Wp, bp)
    return out


# ---------------------------------------------------------------------------
# NTFF profiling support (test harness only; not needed for kernel()).
# The container's axon PJRT .so exposes start/stop NRT-profile entry points;
# drive them directly via ctypes and post-process with gauge.
# ---------------------------------------------------------------------------

def _ntff_hook():
    import contextlib
    import ctypes

    lib = ctypes.CDLL("/opt/axon/libaxon_pjrt.so")
    lib.axon_start_nrt_profile.argtypes = [
        ctypes.POINTER(ctypes.c_int64),
        ctypes.c_size_t,
    ]
    lib.axon_start_nrt_profile.restype = ctypes.c_int64
    lib.axon_stop_nrt_profile.argtypes = [ctypes.c_char_p]
    lib.axon_stop_nrt_profile.restype = ctypes.c_int64

    @contextlib.contextmanager
    def _hook(output_dir, device_ids):
        import jax

        jax.devices()
        if device_ids:
            ids = (ctypes.c_int64 * len(device_ids))(*device_ids)
            rc = lib.axon_start_nrt_profile(ids, len(device_ids))
        else:
            rc = lib.axon_start_nrt_profile(None, 0)
        if rc != 0:
            raise RuntimeError(f"axon_start_nrt_profile rc={rc}")
        try:
            yield
        finally:
            n = lib.axon_stop_nrt_profile(str(output_dir).encode())
            print(f"profile: {n} file(s) written to {output_dir}")

    return _hook


def run_traced(x, Wq, Wk, Wv, Wp, bp, outdir=None, cores=(0,)):
    """Run once under NTFF profiling; returns (out, exec_time_ns, trace_path)."""
    import glob
    import tempfile

    import gauge.profiler
    from concourse._compat import FishPath

    S = x.shape[1]
    fn, in_names, out_names, out_avals, mesh = _get_runner(S)
    in_maps = prep_inputs(x, Wq, Wk, Wv, Wp, bp)
    args = []
    for nm in in_names:
        if nm in _REPLICATED:
            args.append(in_maps[0][nm])
        else:
            args.append(np.concatenate([in_maps[c][nm] for c in range(NCORES)], axis=0))
    zero_outs = [
        np.zeros((NCORES * av.shape[0], *av.shape[1:]), av.dtype) for av in out_avals
    ]
    # warm (compile + first exec)
    out_arrs = fn(*args, *zero_outs)
    _ = [np.asarray(o) for o in out_arrs]

    if outdir is None:
        outdir = tempfile.mkdtemp(prefix="ntff_")
    hook = _ntff_hook()
    zero_outs = [
        np.zeros((NCORES * av.shape[0], *av.shape[1:]), av.dtype) for av in out_avals
    ]
    with hook(outdir, list(cores)):
        out_arrs = fn(*args, *zero_outs)
        out_np = [np.asarray(o) for o in out_arrs]

    ntffs = glob.glob(f"{outdir}/*.ntff")
    if not ntffs:
        print(f"no NTFF files in {outdir}")
        return None, None, None
    nc = get_nc(S)
    profile = gauge.profiler.Profile(
        profile_path=FishPath(outdir),
        kernel_dev_mode=True,
        profile_on_exit=False,
        bass_kernel=nc.m,
        offline_processing=True,
        fname="*_body*",
        metadata={"artifacts_path": outdir},
    )
    results = profile.to_perfetto(model_index=tuple(range(len(cores))))
    exec_ns = max(r.exec_time_ns for r in results)
    yfull = _assemble_y(
        [out_np[out_names.index("y")].reshape(NCORES, -1, D)[c] for c in range(NCORES)]
    )
    return yfull, exec_ns, results[0].trace_path


# revision 25
# speedup vs baseline: 1.0225x; 1.0225x over previous
"""Trainium2 Bass kernel for causal multi-head attention + output projection.

Problem: B=2, S=2048, D=1024, H=16 heads of HD=64; fp32; causal softmax
scaled by D**-0.5; output projection with bias.

Sharding: 2 heads per core (tensor parallel on heads) for QKV + attention,
then an on-device AllToAll reshards from head-split to sequence-split and
each core computes its 512 rows of the output projection locally.

Math notes:
 - All attention tensors are kept transposed ([feature, seq] layouts) so
   every matmul contracts on the partition dim with zero on-chip transposes
   (except V, which is produced as V^T and transposed via the PE).
 - softmax is computed without max-subtraction: logits are N(0, 1/16) by
   construction (scale = 1/32 over a 64-dim dot of unit-variance q,k), so
   exp() is numerically safe; the denominator is accumulated by a column of
   ones appended to V (row 64 of the O^T PSUM accumulator).
 - float32r (TF32-like) matmuls run at bf16 rate with ~1e-4 relative error.
"""

import sys

sys.path.insert(0, "/opt/trn_rl_repo")

import numpy as np

import concourse.bacc as bacc
import concourse.mybir as mybir
import concourse.tile as tile
from concourse.bass_utils import run_bass_kernel_spmd
B, D, H, HD = 2, 1024, 16, 64
NCORES = 8
SCALE = float(D) ** -0.5
F32 = mybir.dt.float32
F32R = mybir.dt.float32r
BF16 = mybir.dt.bfloat16
Exp = mybir.ActivationFunctionType.Exp


def build(S=2048, dump=False):
    KD = D // 128          # contraction tiles for the projections
    NT = S // 128          # key tiles
    SQ = 512               # query-chunk width
    NCH = S // SQ          # query chunks per (batch, head)
    HSL = S // NCORES      # rows of output owned per core per batch

    nc = bacc.Bacc("TRN2", target_bir_lowering=False, debug=False)
    xT = nc.dram_tensor("xT", [B, D, S], BF16, kind="ExternalInput")
    Wqkv = nc.dram_tensor("Wqkv", [128, 3, D // 128, 128], BF16, kind="ExternalInput")
    WpT = nc.dram_tensor("WpT", [128, D // 128, D], BF16, kind="ExternalInput")
    bp = nc.dram_tensor("bp", [1, D], BF16, kind="ExternalInput")
    mask = nc.dram_tensor("mask", [128, 128], BF16, kind="ExternalInput")
    idin = nc.dram_tensor("idin", [128, 128], BF16, kind="ExternalInput")
    sel = nc.dram_tensor("sel", [16, KD, 128], F32, kind="ExternalInput")
    # y rows: [0:HSL] = batch0 s-slice, [HSL:2*HSL] = batch1 s-slice
    y = nc.dram_tensor("y", [B * HSL, D], F32, kind="ExternalOutput")

    with tile.TileContext(nc) as tc:
        ctx_pools = [
            tc.tile_pool(name="persist", bufs=1),
            tc.tile_pool(name="dram", bufs=1, space="DRAM"),
            tc.tile_pool(name="wq", bufs=1),
            tc.tile_pool(name="xp", bufs=2),
            tc.tile_pool(name="qk", bufs=2),
            tc.tile_pool(name="vp", bufs=2),
            tc.tile_pool(name="at", bufs=6),
            tc.tile_pool(name="prj", bufs=2),
            tc.tile_pool(name="yo", bufs=2),
            tc.tile_pool(name="ps_mix", bufs=2, space="PSUM"),
            tc.tile_pool(name="ps_sc", bufs=2, space="PSUM"),
            tc.tile_pool(name="ps_oT", bufs=2, space="PSUM"),
        ]
        import contextlib

        with contextlib.ExitStack() as stk:
            (
                persist, dram, wpool, xpool, qkpool, vppool, atpool,
                prjpool, ypool, ps_mix, ps_sc, ps_oT,
            ) = [stk.enter_context(p) for p in ctx_pools]

            # ---- critical-path first: small constants, weights, batch-0 x ----
            ident = persist.tile([128, 128], BF16)
            nc.sync.dma_start(out=ident, in_=idin[:, :])
            mask_sb = persist.tile([128, 128], BF16)
            nc.sync.dma_start(out=mask_sb, in_=mask[:, :])
            wqkv_sb = wpool.tile([128, 3, KD, 128], BF16)
            nc.sync.dma_start(out=wqkv_sb, in_=Wqkv[:, :, :, :])
            # PE warm-up while input DMAs are in flight: ramps the clock gate
            wps = ps_sc.tile([128, 2, SQ], F32, tag="ps_sc", name="warmps")
            for _ in range(90):
                nc.tensor.matmul(wps[:, 0, 0:128], ident, ident, start=True, stop=True)

            def load_x(b):
                xs = [
                    xpool.tile([128, S], BF16, tag=f"x{t}", name=f"x_{b}_{t}")
                    for t in range(KD)
                ]
                for t in range(KD):
                    nc.sync.dma_start(
                        out=xs[t], in_=xT[b, 128 * t : 128 * (t + 1), :]
                    )
                return xs

            x_sb = {0: load_x(0)}

            ones_sb = persist.tile([1, 128], BF16)
            nc.vector.memset(ones_sb, 1.0)

            oT_sb = {
                (b, hs): persist.tile(
                    [65, S], F32, tag=f"oT_{b}_{hs}", name=f"oT_{b}_{hs}"
                )
                for b in range(B)
                for hs in range(2)
            }
            a2a_in = {
                (b, hs): dram.tile([NCORES, 65, HSL], F32, name=f"a2a_in_{b}_{hs}")
                for b in range(B)
                for hs in range(2)
            }
            a2a_out = {
                (b, hs): dram.tile([NCORES, 65, HSL], F32, name=f"a2a_out_{b}_{hs}")
                for b in range(B)
                for hs in range(2)
            }
            qkvT = {}
            vp = {}

            def emit_qkv_group(b, w, n, eng=None):
                if b not in qkvT:
                    qkvT[b] = qkpool.tile(
                        [128, 3, S], BF16, tag="qkvT", name=f"qkvT_{b}"
                    )
                ps = ps_mix.tile([128, SQ], F32, tag="mix", name=f"psqk_{b}_{w}_{n}")
                for t in range(KD):
                    nc.tensor.matmul(
                        ps,
                        wqkv_sb[:, w, t, :],
                        x_sb[b][t][:, SQ * n : SQ * (n + 1)],
                        start=(t == 0),
                        stop=(t == KD - 1),
                    )
                dst = qkvT[b][:, w, SQ * n : SQ * (n + 1)]
                if eng == "scalar":
                    nc.scalar.copy(dst, ps)
                else:
                    nc.vector.tensor_copy(dst, ps)

            def emit_v_unit(b, i):
                if b not in vp:
                    vp[b] = vppool.tile(
                        [128, NT, 2, 65], BF16, tag="vp", name=f"vp_{b}"
                    )
                    nc.vector.memset(vp[b][:, :, :, 64], 1.0)
                pst = ps_mix.tile([128, 128], BF16, tag="mix", name=f"psvt_{b}_{i}")
                nc.tensor.transpose(
                    pst, qkvT[b][:, 2, 128 * i : 128 * (i + 1)], ident[:, :]
                )
                for hs in range(2):
                    nc.vector.tensor_copy(
                        vp[b][:, i, hs, 0:64], pst[:, 64 * hs : 64 * hs + 64]
                    )

            def emit_attn_chunk(b, hs, n, fillers, stride=2):
                qT = qkvT[b][64 * hs : 64 * hs + 64, 0, :]
                kT = qkvT[b][64 * hs : 64 * hs + 64, 1, :]
                ot = ps_oT.tile(
                    [65, SQ], F32, tag="ps_oT", name=f"ot_{b}_{hs}_{n}"
                )
                jmax = 4 * n + 4
                for jp in range(0, jmax, 2):
                    sc = ps_sc.tile(
                        [128, 2, SQ], F32, tag="ps_sc", name=f"sc_{b}_{hs}_{n}_{jp}"
                    )
                    at = atpool.tile([128, 2, SQ], BF16, tag="at")
                    offs = []
                    for k in range(2):
                        j = jp + k
                        off = max(0, 128 * j - SQ * n)
                        offs.append(off)
                        nc.tensor.matmul(
                            sc[:, k, off:],
                            kT[:, 128 * j : 128 * (j + 1)],
                            qT[:, SQ * n + off : SQ * (n + 1)],
                            start=True,
                            stop=True,
                        )
                    # one exp over both halves (covers a dead zone between them)
                    o0 = offs[0]
                    nc.scalar.activation(
                        at[:, :, :].rearrange("p a s -> p (a s)")[:, o0:],
                        sc[:, :, :].rearrange("p a s -> p (a s)")[:, o0:],
                        Exp,
                        scale=SCALE,
                    )
                    for k in range(2):
                        j = jp + k
                        off = offs[k]
                        if j >= 4 * n:
                            nc.gpsimd.tensor_mul(
                                at[:, k, off : off + 128],
                                at[:, k, off : off + 128],
                                mask_sb,
                            )
                        nc.tensor.matmul(
                            ot[:, off:],
                            vp[b][:, j, hs, :],
                            at[:, k, off:],
                            start=(j == 0),
                            stop=(j == jmax - 1),
                        )
                    if fillers:
                        fillers[0] -= 2
                        if fillers[0] <= 0 and len(fillers) > 1:
                            fillers[0] = fillers.pop(1)
                            fillers.pop(1)()
                nc.vector.tensor_copy(oT_sb[(b, hs)][:, SQ * n : SQ * (n + 1)], ot)

            def emit_staging_cc(b, hs):
                for d in range(NCORES):
                    s0 = d * HSL
                    nc.sync.dma_start(
                        out=a2a_in[(b, hs)][d, :, :],
                        in_=oT_sb[(b, hs)][:, s0 : s0 + HSL],
                    )
                nc.gpsimd.collective_compute(
                    "AllToAll",
                    mybir.AluOpType.bypass,
                    replica_groups=[list(range(NCORES))],
                    ins=[a2a_in[(b, hs)][:, :, :].opt()],
                    outs=[a2a_out[(b, hs)][:, :, :].opt()],
                )

            # ---- phase D (output projection) pieces for batch b ----
            dstate = {}

            def emit_D_head(b):
                st_ = {}
                st_["den"] = prjpool.tile([16, HSL], F32, tag="den", name=f"den_{b}")
                for h_ in range(2):
                    nc.sync.dma_start(
                        out=st_["den"][8 * h_ : 8 * h_ + 8, :],
                        in_=a2a_out[(b, h_)][:, 64, :],
                    )
                st_["rcp"] = prjpool.tile([16, HSL], F32R, tag="rcp", name=f"rcp_{b}")
                with nc.allow_low_precision(reason="softmax denom recip"):
                    nc.vector.reciprocal(st_["rcp"], st_["den"][:, :].bitcast(F32R))
                st_["onrm"] = prjpool.tile(
                    [128, KD, HSL], BF16, tag="onrm", name=f"onrm_{b}"
                )
                st_["accs"] = {}
                dstate[b] = st_

            def emit_D_norm(b, t):
                st_ = dstate[b]
                orc = prjpool.tile([128, HSL], F32, tag="orc", name=f"orc_{b}_{t}")
                for h_ in range(2):
                    nc.sync.dma_start(
                        out=orc[64 * h_ : 64 * h_ + 64, :],
                        in_=a2a_out[(b, h_)][t, 0:64, :],
                    )
                bc = ps_mix.tile([128, HSL], F32, tag="mix", name=f"bc_{b}_{t}")
                nc.tensor.matmul(bc, sel_sb[:, t, :], st_["rcp"], start=True, stop=True)
                nc.vector.tensor_mul(st_["onrm"][:, t, :], orc, bc)

            def emit_D_group(b, st, nn):
                st_ = dstate[b]
                acc = ps_mix.tile(
                    [128, 512], F32, tag="mix", name=f"acc_{b}_{st}_{nn}"
                )
                for t in range(KD):
                    nc.tensor.matmul(
                        acc,
                        st_["onrm"][:, t, 128 * st : 128 * (st + 1)],
                        wpT_sb[:, t, 512 * nn : 512 * (nn + 1)],
                        start=(t == 0),
                        stop=False,
                    )
                nc.tensor.matmul(
                    acc, ones_sb, bp_sb[:, 512 * nn : 512 * (nn + 1)],
                    start=False, stop=True,
                )
                ys = st_.setdefault("y", {})
                if st not in ys:
                    ys[st] = ypool.tile(
                        [128, D], F32, tag="y", name=f"y_{b}_{st}"
                    )
                if b == 1:
                    nc.scalar.copy(ys[st][:, 512 * nn : 512 * (nn + 1)], acc)
                else:
                    nc.vector.tensor_copy(ys[st][:, 512 * nn : 512 * (nn + 1)], acc)
                if nn == D // 512 - 1:
                    nc.sync.dma_start(
                        out=y[b * HSL + 128 * st : b * HSL + 128 * (st + 1), :],
                        in_=ys[st],
                    )

            # ---- schedule ----
            # batch 0 projections (dense PE, warms HAM)
            for w in range(3):
                for n in range(NCH):
                    emit_qkv_group(0, w, n, eng="scalar")
            for i in range(NT):
                emit_v_unit(0, i)
            # batch-1 x load starts now (behind batch-0 x on the DMA queues)
            x_sb[1] = load_x(1)

            # batch-0 attention with batch-1 projection woven in
            units = []
            for w in range(3):
                for n in range(NCH):
                    units.append(lambda w=w, n=n: emit_qkv_group(1, w, n))
            for i in range(NT):
                units.append(lambda i=i: emit_v_unit(1, i))
            total_j = 2 * sum(4 * n + 4 for n in range(NCH))
            pace = max(2, total_j // max(len(units), 1))
            fillers = [pace]
            for u in units:
                fillers.extend([pace, u])
            for hs in range(2):
                for n in range(NCH):
                    emit_attn_chunk(0, hs, n, fillers)
                emit_staging_cc(0, hs)
            for k in range(2, len(fillers), 2):
                fillers[k]()

            # deferred big loads for phase D (transfer during batch-1 attention)
            wpT_sb = persist.tile([128, KD, D], BF16)
            nc.sync.dma_start(out=wpT_sb, in_=WpT[:, :, :])
            bp_sb = persist.tile([1, D], BF16)
            nc.sync.dma_start(out=bp_sb, in_=bp[:, :])
            sel_sb = persist.tile([16, KD, 128], F32R)
            nc.sync.dma_start(out=sel_sb, in_=sel[:, :, :].bitcast(F32R))

            # batch-1 attention with phase-D(batch 0) woven in
            units = [lambda: emit_D_head(0)]
            for t in range(KD):
                units.append(lambda t=t: emit_D_norm(0, t))
            for st in range(HSL // 128):
                for nn in range(D // 512):
                    units.append(lambda st=st, nn=nn: emit_D_group(0, st, nn))
            chunks1 = [(hs, n) for hs in range(2) for n in range(NCH)]
            start_ci = len(chunks1) // 2 + 1
            late = chunks1[start_ci:]
            total_j = 2 * sum(4 * n + 4 for _, n in late)
            pace = max(2, total_j // max(len(units) - 1, 1))
            fillers = [pace]
            for u in units[1:]:
                fillers.extend([pace, u])
            for ci, (hs, n) in enumerate(chunks1):
                if ci == start_ci - 1:
                    units[0]()  # D0 head: den DMAs + reciprocal
                emit_attn_chunk(1, hs, n, fillers if ci >= start_ci else None)
                if n == NCH - 1:
                    emit_staging_cc(1, hs)
            for k in range(2, len(fillers), 2):
                fillers[k]()

            # phase D for batch 1 (tail)
            emit_D_head(1)
            for t in range(KD):
                emit_D_norm(1, t)
            for st in range(HSL // 128):
                for nn in range(D // 512):
                    emit_D_group(1, st, nn)

    nc.compile()
    return nc


_built = {}


def get_nc(S=2048):
    if S not in _built:
        _built[S] = build(S)
    return _built[S]


def prep_inputs(x, Wq, Wk, Wv, Wp, bp):
    """Host-side shard prep. Returns per-core input maps."""
    import ml_dtypes

    BF = ml_dtypes.bfloat16
    x = np.ascontiguousarray(np.asarray(x, dtype=np.float32))
    Wq, Wk, Wv = (np.asarray(w, dtype=np.float32) for w in (Wq, Wk, Wv))
    Wp = np.asarray(Wp, dtype=np.float32)
    bp = np.asarray(bp, dtype=np.float32)
    BFc = BF
    xT = np.ascontiguousarray(x.transpose(0, 2, 1)).astype(BFc)
    KD = D // 128
    # WpT pre-arranged for SBUF: [p, t, i] with row t*128+p of Wp.T
    WpT = np.ascontiguousarray(
        Wp.T.reshape(KD, 128, D).transpose(1, 0, 2)
    ).astype(BFc)
    mask = np.triu(np.ones((128, 128), dtype=np.float32)).astype(BFc)
    idin = np.eye(128, dtype=np.float32).astype(BFc)
    sel = np.zeros((16, KD, 128), dtype=np.float32)
    for t in range(KD):
        sel[t, t, 0:64] = 1.0          # head 2t     -> den row 0*8 + t
        sel[8 + t, t, 64:128] = 1.0    # head 2t + 1 -> den row 1*8 + t
    in_maps = []
    for c in range(NCORES):
        h0 = 2 * c
        wqkv = np.stack(
            [
                np.concatenate([Wq[h0], Wq[h0 + 1]], axis=1),
                np.concatenate([Wk[h0], Wk[h0 + 1]], axis=1),
                np.concatenate([Wv[h0], Wv[h0 + 1]], axis=1),
            ]
        )  # [3, D, 128]
        # pre-arrange: [p, w, t, m]
        wqkv = np.ascontiguousarray(
            wqkv.reshape(3, KD, 128, 128).transpose(2, 0, 1, 3)
        ).astype(BF)
        in_maps.append(
            {
                "xT": xT,
                "Wqkv": wqkv,
                "WpT": WpT,
                "bp": bp.reshape(1, D).astype(BF),
                "mask": mask,
                "idin": idin,
                "sel": sel,
            }
        )
    return in_maps


# inputs identical across cores are passed replicated (shipped once, not 8x)
_REPLICATED = {"xT", "WpT", "bp", "mask", "idin"}

_runners = {}


def _get_runner(S):
    """Cached jitted SPMD callable for the built module."""
    if S in _runners:
        return _runners[S]
    import jax
    import concourse.mybir as _mybir
    from concourse import bass2jax
    from jax.experimental.shard_map import shard_map
    from jax.sharding import Mesh, PartitionSpec

    nc = get_nc(S)
    bass2jax.install_neuronx_cc_hook()

    in_names, out_names, out_avals = [], [], []
    partition_name = nc.partition_id_tensor.name if nc.partition_id_tensor else None
    for alloc in nc.m.functions[0].allocations:
        if not isinstance(alloc, _mybir.MemoryLocationSet):
            continue
        name = alloc.memorylocations[0].name
        if alloc.kind == "ExternalInput":
            if name != partition_name:
                in_names.append(name)
        elif alloc.kind == "ExternalOutput":
            out_names.append(name)
            out_avals.append(
                jax.core.ShapedArray(tuple(alloc.tensor_shape), _mybir.dt.np(alloc.dtype))
            )
    n_params = len(in_names)
    all_in_names = list(in_names) + list(out_names)
    if partition_name is not None:
        all_in_names.append(partition_name)

    def _body(*args):
        operands = list(args)
        if partition_name is not None:
            operands.append(bass2jax.partition_id_tensor())
        outs = bass2jax._bass_exec_p.bind(
            *operands,
            out_avals=tuple(out_avals),
            in_names=tuple(all_in_names),
            out_names=tuple(out_names),
            lowering_input_output_aliases=(),
            sim_require_finite=True,
            sim_require_nnan=True,
            nc=nc,
        )
        return tuple(outs)

    devices = jax.devices()[:NCORES]
    mesh = Mesh(np.asarray(devices), ("core",))
    in_specs = tuple(
        PartitionSpec() if nm in _REPLICATED else PartitionSpec("core")
        for nm in in_names
    ) + (PartitionSpec("core"),) * len(out_names)
    out_specs = (PartitionSpec("core"),) * len(out_names)
    donate = tuple(range(n_params, n_params + len(out_names)))
    fn = jax.jit(
        shard_map(_body, mesh=mesh, in_specs=in_specs, out_specs=out_specs, check_rep=False),
        donate_argnums=donate,
        keep_unused=True,
    )
    r = (fn, in_names, out_names, out_avals, mesh)
    _runners[S] = r
    return r


class _Res:
    def __init__(self, results):
        self.results = results
        self.exec_time_ns = None


def run(x, Wq, Wk, Wv, Wp, bp, timings=None):
    import time as _time

    S = x.shape[1]
    t0 = _time.perf_counter()
    fn, in_names, out_names, out_avals, mesh = _get_runner(S)
    t1 = _time.perf_counter()
    in_maps = prep_inputs(x, Wq, Wk, Wv, Wp, bp)
    t2 = _time.perf_counter()
    args = []
    for nm in in_names:
        if nm in _REPLICATED:
            args.append(in_maps[0][nm])
        else:
            args.append(np.concatenate([in_maps[c][nm] for c in range(NCORES)], axis=0))
    zero_outs = [
        np.zeros((NCORES * av.shape[0], *av.shape[1:]), av.dtype) for av in out_avals
    ]
    t3 = _time.perf_counter()
    out_arrs = fn(*args, *zero_outs)
    out_np = [np.asarray(o) for o in out_arrs]
    t4 = _time.perf_counter()
    results = [
        {
            nm: out_np[i].reshape(NCORES, *out_avals[i].shape)[c]
            for i, nm in enumerate(out_names)
        }
        for c in range(NCORES)
    ]
    if timings is not None:
        timings.update(
            runner=t1 - t0, prep=t2 - t1, concat=t3 - t2, exec=t4 - t3
        )
    return _assemble_y([results[c]["y"] for c in range(NCORES)]), _Res(results)


def _assemble_y(per_core):
    """per-core y is [B*HSL, D]: rows [b*HSL:(b+1)*HSL] = batch b, s-slice c."""
    HSL = per_core[0].shape[0] // B
    S = HSL * NCORES
    out = np.empty((B, S, D), dtype=per_core[0].dtype)
    for c in range(NCORES):
        for b in range(B):
            out[b, HSL * c : HSL * (c + 1), :] = per_core[c][b * HSL : (b + 1) * HSL]
    return out


def kernel(x, Wq, Wk, Wv, Wp, bp):
    out, _ = run(x, Wq, Wk, Wv, Wp, bp)
    return out


# ---------------------------------------------------------------------------
# NTFF profiling support (test harness only; not needed for kernel()).
# The container's axon PJRT .so exposes start/stop NRT-profile entry points;
# drive them directly via ctypes and post-process with gauge.
# ---------------------------------------------------------------------------

def _ntff_hook():
    import contextlib
    import ctypes

    lib = ctypes.CDLL("/opt/axon/libaxon_pjrt.so")
    lib.axon_start_nrt_profile.argtypes = [
        ctypes.POINTER(ctypes.c_int64),
        ctypes.c_size_t,
    ]
    lib.axon_start_nrt_profile.restype = ctypes.c_int64
    lib.axon_stop_nrt_profile.argtypes = [ctypes.c_char_p]
    lib.axon_stop_nrt_profile.restype = ctypes.c_int64

    @contextlib.contextmanager
    def _hook(output_dir, device_ids):
        import jax

        jax.devices()
        if device_ids:
            ids = (ctypes.c_int64 * len(device_ids))(*device_ids)
            rc = lib.axon_start_nrt_profile(ids, len(device_ids))
        else:
            rc = lib.axon_start_nrt_profile(None, 0)
        if rc != 0:
            raise RuntimeError(f"axon_start_nrt_profile rc={rc}")
        try:
            yield
        finally:
            n = lib.axon_stop_nrt_profile(str(output_dir).encode())
            print(f"profile: {n} file(s) written to {output_dir}")

    return _hook


def run_traced(x, Wq, Wk, Wv, Wp, bp, outdir=None, cores=(0,)):
    """Run once under NTFF profiling; returns (out, exec_time_ns, trace_path)."""
    import glob
    import tempfile

    import gauge.profiler
    from concourse._compat import FishPath

    S = x.shape[1]
    fn, in_names, out_names, out_avals, mesh = _get_runner(S)
    in_maps = prep_inputs(x, Wq, Wk, Wv, Wp, bp)
    args = []
    for nm in in_names:
        if nm in _REPLICATED:
            args.append(in_maps[0][nm])
        else:
            args.append(np.concatenate([in_maps[c][nm] for c in range(NCORES)], axis=0))
    zero_outs = [
        np.zeros((NCORES * av.shape[0], *av.shape[1:]), av.dtype) for av in out_avals
    ]
    # warm (compile + first exec)
    out_arrs = fn(*args, *zero_outs)
    _ = [np.asarray(o) for o in out_arrs]

    if outdir is None:
        outdir = tempfile.mkdtemp(prefix="ntff_")
    hook = _ntff_hook()
    zero_outs = [
        np.zeros((NCORES * av.shape[0], *av.shape[1:]), av.dtype) for av in out_avals
    ]
    with hook(outdir, list(cores)):
        out_arrs = fn(*args, *zero_outs)
        out_np = [np.asarray(o) for o in out_arrs]

    ntffs = glob.glob(f"{outdir}/*.ntff")
    if not ntffs:
        print(f"no NTFF files in {outdir}")
        return None, None, None
    nc = get_nc(S)
    profile = gauge.profiler.Profile(
        profile_path=FishPath(outdir),
        kernel_dev_mode=True,
        profile_on_exit=False,
        bass_kernel=nc.m,
        offline_processing=True,
        fname="*_body*",
        metadata={"artifacts_path": outdir},
    )
    results = profile.to_perfetto(model_index=tuple(range(len(cores))))
    exec_ns = max(r.exec_time_ns for r in results)
    yfull = _assemble_y(
        [out_np[out_names.index("y")].reshape(NCORES, -1, D)[c] for c in range(NCORES)]
    )
    return yfull, exec_ns, results[0].trace_path
